# revision 13
# baseline (speedup 1.0000x reference)
# GATConv Trainium kernel: host prep + Bass program builder (parameterized).
import numpy as np
import ml_dtypes
import concourse.bass as bass
import concourse.bacc as bacc
import concourse.mybir as mybir
import concourse.tile as tile
from concourse._compat import exact_div

F32 = mybir.dt.float32
BF16 = mybir.dt.bfloat16
I16 = mybir.dt.int16
FP8 = mybir.dt.float8e4

ALPHA = 0.2
H, D = 8, 32
HD = H * D            # 256
IN = 256
FT_W = 384            # padded ft row (bf16) -> 768B stride; cols 0:256 ft, 256:264 el bf16
ROW = HD + 8          # 264 gathered cols per edge


def _ceil(a, b):
    return -(-a // b)


class Plan:
    """Host-side uniform schedule shared by all cores.

    Chunk-major schedule: for q in chunks, for s in supertiles, one call of
    densely packed slots (per-tile ranges sized by max-over-cores counts).
    Blocks are 128-slot windows cut across tile boundaries; a (block x tile)
    intersection is a segment (one erx + one agg matmul each). Per-tile
    results accumulate across chunk passes in an SBUF accumulator."""

    def __init__(self, N, E, src, dst, n_cores, tiles_per_core, st_tiles=4,
                 chunk=32768, wmax=8):
        self.N, self.E, self.C = N, E, n_cores
        self.NT = tiles_per_core              # dst tiles per core
        self.ND = tiles_per_core * 128        # dsts per core
        NNfull = self.ND * n_cores
        assert NNfull >= N
        self.chunk = chunk
        self.ST = st_tiles
        self.WMAX = wmax

        order = np.argsort(dst, kind="stable")
        src_s, dst_s = src[order], dst[order]
        core_of = dst_s // self.ND
        tile_of = (dst_s % self.ND) // 128

        # per-core compacted node table: [own dsts (ND rows, tile order) |
        # other distinct srcs sorted]; comp index addresses the ft table.
        self.node_order = []                  # per core: original node id per row
        comp_of = np.empty(E, dtype=np.int64)  # per (sorted) edge: comp idx of src
        used = 0
        for c in range(n_cores):
            sel = core_of == c
            srcs = src_s[sel]
            own_lo, own_hi = c * self.ND, (c + 1) * self.ND
            others = np.unique(srcs)
            others = others[(others < own_lo) | (others >= own_hi)]
            order_c = np.concatenate([np.arange(own_lo, own_hi), others])
            self.node_order.append(order_c)
            used = max(used, len(order_c))
            # comp idx: own -> src-own_lo ; other -> ND + rank in others
            ci = np.where((srcs >= own_lo) & (srcs < own_hi),
                          srcs - own_lo,
                          self.ND + np.searchsorted(others, srcs))
            comp_of[sel] = ci
        self.NN = _ceil(used, 2048) * 2048    # pad to fc-group multiple
        self.NQ = _ceil(self.NN, chunk)
        q_of = comp_of // chunk

        cnt = np.zeros((n_cores, self.NT, self.NQ), dtype=np.int64)
        np.add.at(cnt, (core_of, tile_of, q_of), 1)
        mx = cnt.max(axis=0)                  # [NT, NQ] slots per group
        mx[:, 0] = np.maximum(mx[:, 0], 1)    # every tile appears in chunk 0
        self.maxcnt = mx

        # per-(c,t,q) edge lists (comp src idx, global dst)
        self.edges = [[[None] * self.NQ for _ in range(self.NT)] for _ in range(n_cores)]
        key = ((core_of * self.NT + tile_of) * self.NQ + q_of)
        order2 = np.argsort(key, kind="stable")
        ks = key[order2]
        bounds = np.searchsorted(ks, np.arange(n_cores * self.NT * self.NQ + 1))
        for c in range(n_cores):
            for t in range(self.NT):
                for q in range(self.NQ):
                    k = (c * self.NT + t) * self.NQ + q
                    sel = order2[bounds[k]:bounds[k + 1]]
                    self.edges[c][t][q] = (comp_of[sel], dst_s[sel])

        # calls: chunk-major, supertile-minor; dense slot ranges per tile
        self.n_st = _ceil(self.NT, st_tiles)
        self.call_layout = []   # (q, base_slot, [(t, a, b)] call-local ranges)
        self.windows = []       # (q, [block -> [(t, c0, c1)]], slot_pos)
        self.tot_segs = [0] * self.NT
        self.call_segs = {}     # (q, t) -> segments of tile t in chunk q
        pos = 0
        for q in range(self.NQ):
            for s in range(self.n_st):
                ts = range(s * st_tiles, min((s + 1) * st_tiles, self.NT))
                items = [(t, int(mx[t, q])) for t in ts if mx[t, q] > 0]
                if not items:
                    continue
                L = sum(n for _, n in items)
                nb = _ceil(L, 128)
                segs_by_block = [[] for _ in range(nb)]
                cur = 0
                ranges = []
                for t, n in items:
                    a, b = cur, cur + n
                    cur = b
                    ranges.append((t, a, b))
                    for bi in range(a // 128, (b - 1) // 128 + 1):
                        c0 = max(a, bi * 128) - bi * 128
                        c1 = min(b, (bi + 1) * 128) - bi * 128
                        segs_by_block[bi].append((t, c0, c1))
                        self.tot_segs[t] += 1
                        self.call_segs[(q, t)] = self.call_segs.get((q, t), 0) + 1
                self.call_layout.append((q, pos, ranges))
                for w0 in range(0, nb, wmax):
                    self.windows.append((q, segs_by_block[w0:w0 + wmax],
                                         pos + w0 * 128))
                pos += nb * 128
        self.NBtot = pos
        # stream i16 sizes per window: nseg*128 (two fp8 onehot sets) + wn*8
        self.win_stw = []
        tot = 0
        for q, blocks, p0 in self.windows:
            wn = len(blocks)
            nseg = sum(len(b) for b in blocks)
            stw = nseg * 128 + wn * 8
            self.win_stw.append(stw)
            tot += stw
        self.stream_i16 = tot
        self.max_stw = max(self.win_stw)

    def build_streams(self, c):
        NB = self.NBtot
        idx_ft = np.zeros(NB, dtype=np.int16)
        dstl = np.full(NB, 200.0, dtype=np.float32)
        for q, base, ranges in self.call_layout:
            for t, a, b in ranges:
                s_arr, d_arr = self.edges[c][t][q]
                n = len(s_arr)
                assert n <= b - a
                idx_ft[base + a:base + a + n] = (s_arr - q * self.chunk).astype(np.int16)
                dstl[base + a:base + a + n] = (d_arr - (c * self.ND + t * 128)).astype(np.float32)

        stream = np.zeros((128, self.stream_i16), dtype=np.int16)
        parts = np.arange(128, dtype=np.float32)
        cols = np.arange(128, dtype=np.float32)
        lanes = np.arange(128)
        i16 = lambda a: np.ascontiguousarray(a).view(np.int16)
        off = 0
        for w, (q, blocks, p0) in enumerate(self.windows):
            wn = len(blocks)
            nb = wn * 128
            nseg = sum(len(b) for b in blocks)
            d_seg = dstl[p0:p0 + nb].reshape(wn, 128)
            x_seg = idx_ft[p0:p0 + nb]
            ohT = np.zeros((128, nseg * 128), dtype=ml_dtypes.float8_e4m3)
            ohs = np.zeros((128, nseg * 128), dtype=ml_dtypes.float8_e4m3)
            si = 0
            for j, segs in enumerate(blocks):
                dj = d_seg[j]
                for (t, c0, c1) in segs:
                    m = (lanes >= c0) & (lanes < c1)
                    # ohT[p, i] = in-seg & dstl_i == p   [dst-rows in partitions]
                    ohT[:, si * 128:(si + 1) * 128] = (
                        (dj[None, :] == parts[:, None]) & m[None, :]
                    ).astype(ml_dtypes.float8_e4m3)
                    # oh[p, d] = in-seg & dstl_p == d    [edges in partitions]
                    ohs[:, si * 128:(si + 1) * 128] = (
                        (dj[:, None] == cols[None, :]) & m[:, None]
                    ).astype(ml_dtypes.float8_e4m3)
                    si += 1
            stream[:, off:off + nseg * 64] = i16(ohT)
            stream[:, off + nseg * 64:off + nseg * 128] = i16(ohs)
            ift = np.zeros((128, nb // 16), dtype=np.int16)
            i = np.arange(nb)
            for k in range(8):
                ift[16 * k + i % 16, i // 16] = x_seg
            stream[:, off + nseg * 128:off + nseg * 128 + wn * 8] = ift
            off += self.win_stw[w]
        assert off == self.stream_i16
        return {"stream": stream}


def make_waug(W, attn_l, attn_r):
    """[IN, 272] f32 cols: [W'^T | Ml | Mr]; W' rows in d-major order d*H+h."""
    perm = np.empty(HD, dtype=np.int64)
    for h in range(H):
        for d in range(D):
            perm[d * H + h] = h * D + d
    Wp = W[perm, :]                                   # [256, IN]
    Ml = np.zeros((IN, H), dtype=np.float32)
    Mr = np.zeros((IN, H), dtype=np.float32)
    for h in range(H):
        rows = W[h * D:(h + 1) * D, :]                # [D, IN]
        Ml[:, h] = attn_l[0, h, :] @ rows
        Mr[:, h] = attn_r[0, h, :] @ rows
    return np.concatenate([Wp.T, Ml, Mr], axis=1).astype(np.float32)


def dma_gather_raw(gp, out_ap, in_ap, idxs_ap, num_idxs, elem_size, elem_step,
                   queue_num=0):
    """dma_gather minus the elem_size%256 assert (row stride must be %256B)."""
    stride_bytes = elem_step * mybir.dt.size(in_ap.dtype)
    stride_bytes_256 = exact_div(stride_bytes, 256)
    _in_ap = gp.lower_ap_dma(in_ap, for_custom_bir_dma=True)
    _idxs_ap = gp.lower_ap(idxs_ap)
    _out_ap = gp.lower_ap(out_ap)
    return gp.add_instruction(
        mybir.InstDMAGatherAnt(
            name=gp.bass.get_next_instruction_name(),
            ins=[*_in_ap, _idxs_ap, gp.lower_val_access(gp.to_reg(num_idxs))],
            outs=[_out_ap],
            transpose=False, num_idxs=num_idxs, elem_size=elem_size,
            stride_bytes_256=stride_bytes_256, gen_mode=0, single_packet=True,
            queue_num=queue_num, sbuf_tokens_per_rank=0, sbuf_free_dim_per_rank=0,
            sbuf_free_dim_pad_per_rank=0, sbuf_byte_offset=0,
        )
    )


def build_program(plan, n_cores, fc_mega=16, nq=3):
    """One SPMD Bass program, A/B interleaved by chunk: FC for chunk q is
    emitted just before chunk q's edge windows, sharing one pool scope so
    engines pipeline across phases. Inputs: featT bf16 [IN,NN], waug bf16,
    stream i16. Output: out [128, NT*256] f32 (partition-major)."""
    p = plan
    NN, ND, NT, ST, WMAX = p.NN, p.ND, p.NT, p.ST, p.WMAX
    nc = bacc.Bacc("TRN2", target_bir_lowering=False, debug=False,
                   num_devices=n_cores, num_swdge_queues=nq)

    featT_d = nc.dram_tensor("featT", [IN, NN], BF16, kind="ExternalInput").ap()
    waug_d = nc.dram_tensor("waug", [IN, HD + 16], BF16, kind="ExternalInput").ap()
    stream_d = nc.dram_tensor("stream", [128, p.stream_i16], I16, kind="ExternalInput").ap()
    n_nt = NN // 128
    ft_ts = []
    for qq in range(p.NQ):
        rows = min(p.chunk, NN - qq * p.chunk)
        ft_ts.append(nc.dram_tensor(f"ft_tab{qq}", [rows, FT_W], BF16,
                                    kind="Internal").ap())
    eler_t = nc.dram_tensor("eler_tab", [128, NT * H], BF16, kind="Internal").ap()
    out_d = nc.dram_tensor("out", [128, NT * HD], F32, kind="ExternalOutput").ap()

    MG = fc_mega
    assert p.chunk % (MG * 128) == 0
    gpc = p.chunk // (MG * 128)        # fc groups per full chunk

    # windows grouped by chunk
    wins_by_q = [[] for _ in range(p.NQ)]
    for w, (q, blocks, pos) in enumerate(p.windows):
        wins_by_q[q].append((w, blocks))

    with tile.TileContext(nc) as tc:
        with tc.tile_pool(name="fca", bufs=2) as apool, \
             tc.tile_pool(name="fcc", bufs=1) as cpool, \
             tc.tile_pool(name="eb", bufs=4) as pool, \
             tc.tile_pool(name="ebs", bufs=6) as stpool, \
             tc.tile_pool(name="ebo", bufs=2) as opool, \
             tc.tile_pool(name="fcp", bufs=2, space="PSUM") as fpsp, \
             tc.tile_pool(name="ebp", bufs=ST, space="PSUM") as psp, \
             tc.tile_pool(name="ebx", bufs=2, space="PSUM") as psx:
            wa = cpool.tile([128, 2, HD + 16], BF16)
            nc.sync.dma_start(wa[:], waug_d.rearrange("(k p) c -> p k c", p=128))
            acc = cpool.tile([128, NT, HD], BF16)      # cross-chunk numerator acc
            accd = cpool.tile([128, NT, H], F32)       # denominator acc (f32)
            er_all = cpool.tile([128, NT, H], BF16)

            agg = {}
            issued = {t: 0 for t in range(NT)}
            issued_q = {}
            ost = {}
            stream_off = {}
            off = 0
            for w in range(len(p.windows)):
                stream_off[w] = off
                off += p.win_stw[w]

            def fc_chunk(qq):
                g0q = qq * (p.chunk // 128)
                for lg in range(0, min(p.chunk // 128, n_nt - g0q), MG):
                    g0 = g0q + lg
                    gn = min(MG, n_nt - g0)
                    ftin = apool.tile([128, 2, MG * 128], BF16, tag="ftin")
                    nc.sync.dma_start(
                        ftin[:, :, :gn * 128],
                        featT_d.rearrange("(k p) n -> p k n", p=128)[:, :, g0 * 128:(g0 + gn) * 128])
                    ftst = apool.tile([128, MG, ROW], BF16, tag="ftst")
                    elst = apool.tile([128, MG, H], BF16, tag="elst")
                    write_el = qq == 0 and g0 < NT
                    for j in range(gn):
                        fc_ps = fpsp.tile([128, HD + 16], F32, tag="fc")
                        for k in range(2):
                            nc.tensor.matmul(fc_ps[:], ftin[:, k, j * 128:(j + 1) * 128],
                                             wa[:, k, :], start=(k == 0), stop=(k == 1))
                        if j % 2 == 0:
                            nc.vector.tensor_copy(ftst[:, j, :], fc_ps[:, 0:ROW])
                        else:
                            nc.scalar.copy(ftst[:, j, :], fc_ps[:, 0:ROW])
                        if write_el and g0 + j < NT:
                            nc.scalar.copy(elst[:, j, :], fc_ps[:, HD + 8:HD + 16])
                    nc.sync.dma_start(
                        ft_ts[qq].rearrange("(g p) c -> p g c", p=128)[:, lg:lg + gn, 0:ROW],
                        ftst[:, :gn, :])
                    if write_el:
                        en = min(gn, NT - g0)
                        nc.scalar.dma_start(eler_t[:, g0 * H:(g0 + en) * H],
                                            elst[:, :en, :])

            for q in range(p.NQ):
                fc_chunk(q)
                if q == 0:
                    nc.scalar.dma_start(
                        er_all[:],
                        eler_t[:].rearrange("p (g c) -> p g c", c=H))
                for wcount, blocks in wins_by_q[q]:
                    wn = len(blocks)
                    NB = wn * 128
                    nseg = sum(len(b) for b in blocks)
                    stw = p.win_stw[wcount]
                    soff = stream_off[wcount]
                    st = stpool.tile([128, p.max_stw], I16, tag="st")
                    nc.sync.dma_start(st[:, :stw], stream_d[:, soff:soff + stw])
                    ohT = st[:, :nseg * 64].bitcast(FP8)
                    ohs = st[:, nseg * 64:nseg * 128].bitcast(FP8)
                    ift = st[:, nseg * 128:nseg * 128 + wn * 8]

                    g = pool.tile([128, WMAX, ROW], BF16, tag="g")
                    dma_gather_raw(nc.gpsimd, g[:, :wn, :],
                                   ft_ts[q][:, 0:ROW],
                                   ift[:], NB, ROW, FT_W,
                                   queue_num=wcount % nq)

                    # er per edge via one-hot matmul (ohT: [dst-rows, edges] fp8)
                    erx_ps = psx.tile([128, WMAX * H], F32, tag="erx",
                                      name=f"erx{wcount}")
                    si = 0
                    for j, segs in enumerate(blocks):
                        for k, (t, c0, c1) in enumerate(segs):
                            nc.tensor.matmul(erx_ps[:, j * H:(j + 1) * H],
                                             ohT[:, si * 128:(si + 1) * 128],
                                             er_all[:, t, :],
                                             start=(k == 0), stop=(k == len(segs) - 1),
                                             skip_group_check=True)
                            si += 1
                    lw = pool.tile([128, WMAX, H], F32, tag="lw")
                    nc.vector.tensor_tensor(
                        lw[:, :wn, :], g[:, :wn, HD:HD + 8],
                        erx_ps.rearrange("p (b h) -> p b h", h=H)[:, :wn, :],
                        mybir.AluOpType.add)
                    nc.vector.scalar_tensor_tensor(lw[:, :wn, :], lw[:, :wn, :],
                                                   ALPHA, lw[:, :wn, :],
                                                   mybir.AluOpType.mult,
                                                   mybir.AluOpType.max)
                    rhs = pool.tile([128, WMAX, HD + 8], BF16, tag="rhs")
                    nc.scalar.activation(rhs[:, :wn, HD:HD + 8], lw[:, :wn, :],
                                         mybir.ActivationFunctionType.Exp)
                    nc.vector.tensor_tensor(
                        rhs[:, :wn, 0:HD].rearrange("p b (d h) -> p b d h", h=H),
                        g[:, :wn, 0:HD].rearrange("p b (d h) -> p b d h", h=H),
                        rhs[:, :wn, HD:HD + 8].unsqueeze(2)
                            .broadcast_to([128, wn, D, H]),
                        mybir.AluOpType.mult)

                    si = 0
                    for j, segs in enumerate(blocks):
                        for (t, c0, c1) in segs:
                            if t not in agg:
                                agg[t] = psp.tile([128, HD + 8], F32, tag="agg",
                                                  name=f"agg{q}_{t}")
                                issued_q[t] = 0
                            at = agg[t]
                            cs = p.call_segs[(q, t)]
                            nc.tensor.matmul(at[:], ohs[:, si * 128:(si + 1) * 128],
                                             rhs[:, j, :],
                                             start=(issued_q[t] == 0),
                                             stop=(issued_q[t] == cs - 1),
                                             skip_group_check=True)
                            issued_q[t] += 1
                            issued[t] += 1
                            si += 1
                            if issued_q[t] == cs:
                                # fold chunk-partial into SBUF accumulators
                                if q == 0:
                                    nc.vector.tensor_copy(acc[:, t, :], at[:, 0:HD])
                                    nc.vector.tensor_copy(accd[:, t, :], at[:, HD:HD + 8])
                                else:
                                    nc.vector.tensor_tensor(
                                        acc[:, t, :], acc[:, t, :], at[:, 0:HD],
                                        mybir.AluOpType.add)
                                    nc.vector.tensor_tensor(
                                        accd[:, t, :], accd[:, t, :], at[:, HD:HD + 8],
                                        mybir.AluOpType.add)
                                del agg[t]
                                del issued_q[t]
                            if issued[t] == p.tot_segs[t]:
                                s = t // ST
                                if s not in ost:
                                    ost[s] = opool.tile([128, ST, HD], F32,
                                                        tag="ost", name=f"ost{s}")
                                pool_ost = ost[s]
                                dsum = pool.tile([128, H], F32, tag="dsum")
                                nc.vector.tensor_scalar(dsum[:], accd[:, t, :],
                                                        1e-20, None,
                                                        mybir.AluOpType.max)
                                recd = pool.tile([128, H], F32, tag="recd")
                                nc.vector.reciprocal(recd[:], dsum[:])
                                nc.vector.tensor_tensor(
                                    pool_ost[:, t % ST, :].rearrange(
                                        "p (h d) -> p h d", d=D),
                                    acc[:, t, :].rearrange("p (d h) -> p h d", h=H),
                                    recd[:].unsqueeze(2).broadcast_to([128, H, D]),
                                    mybir.AluOpType.mult)
                                t0 = s * ST
                                n_in_st = min(ST, NT - t0)
                                if all(issued[tt] == p.tot_segs[tt]
                                       for tt in range(t0, t0 + n_in_st)):
                                    nc.scalar.dma_start(
                                        out_d[:, t0 * HD:(t0 + n_in_st) * HD]
                                        .rearrange("p (g c) -> p g c", c=HD),
                                        pool_ost[:, :n_in_st, :])
                                    del ost[s]
    return _finish(nc)


def _finish(nc):
    nc.compile()
    return nc


def host_prep(feat, W, attn_l, attn_r, src, dst, n_cores, tiles_per_core,
              st_tiles=4, chunk=32768, wmax=8):
    N = feat.shape[0]
    E = src.shape[0]
    plan = Plan(N, E, src.astype(np.int64), dst.astype(np.int64), n_cores,
                tiles_per_core, st_tiles, chunk, wmax)
    featb = np.zeros((plan.ND * n_cores, IN), dtype=ml_dtypes.bfloat16)
    featb[:N] = feat.astype(ml_dtypes.bfloat16)
    waug = make_waug(W, attn_l, attn_r).astype(ml_dtypes.bfloat16)
    in_maps = []
    for c in range(n_cores):
        s = plan.build_streams(c)
        ftc = np.zeros((IN, plan.NN), dtype=ml_dtypes.bfloat16)
        oc = plan.node_order[c]
        ftc[:, :len(oc)] = featb[oc].T
        in_maps.append({
            "featT": ftc, "waug": waug,
            "stream": s["stream"],
        })
    return plan, in_maps


def assemble_output(plan, results, N):
    full = np.zeros((plan.ND * plan.C, HD), dtype=np.float32)
    for c in range(plan.C):
        r = results[c]["out"].reshape(128, plan.NT, HD)
        full[c * plan.ND:(c + 1) * plan.ND] = (
            r.transpose(1, 0, 2).reshape(plan.ND, HD))
    return full[:N].reshape(N, H, D)


# ----------------------------------------------------------------------------
# Harness entrypoint: full inputs in, full output out. Shapes hardcoded for
# nn_GATConv (N=100000, E=1600000, IN=256, H=8, D=32) on 8 NeuronCores.
# ----------------------------------------------------------------------------
from concourse.bass_interp import get_hw_module as _get_hw_module
from concourse import bass_utils as _bass_utils

_N_CORES = 8
_TPC = 98            # dst tiles per core (98*128*8 = 100352 >= 100000)
_ST_TILES = 4
_CHUNK = 32768
_WMAX = 8
_NQ = 3              # SWDGE queues: windows alternate queues

_cache = {}


def kernel(feat, W, attn_l, attn_r, src, dst):
    feat = np.ascontiguousarray(np.asarray(feat, dtype=np.float32))
    W = np.ascontiguousarray(np.asarray(W, dtype=np.float32))
    attn_l = np.asarray(attn_l, dtype=np.float32)
    attn_r = np.asarray(attn_r, dtype=np.float32)
    src = np.asarray(src).astype(np.int64)
    dst = np.asarray(dst).astype(np.int64)
    N = feat.shape[0]

    plan, in_maps = host_prep(feat, W, attn_l, attn_r, src, dst,
                              _N_CORES, _TPC, st_tiles=_ST_TILES,
                              chunk=_CHUNK, wmax=_WMAX)
    key = "prog"
    if key not in _cache:
        nc = build_program(plan, _N_CORES, nq=_NQ)
        nc.m = _get_hw_module(nc.m)
        _cache[key] = nc
    nc = _cache[key]
    res = _bass_utils.run_bass_kernel_spmd(nc, in_maps,
                                           core_ids=list(range(_N_CORES)))
    return assemble_output(plan, res.results, N)
